# revision 7
# baseline (speedup 1.0000x reference)
"""Causal self-attention (B=4, T=2048, C=1024, H=16) on 8 TRN2 NeuronCores.

Sharding: core = 2*b + g  (b = batch 0..3, g = head-group of 8 heads).
v2: full f16 datapath (f32 PSUM accumulation), x resident in SBUF,
chunk-major attention, per-chunk partial output projection, and a
pairwise ReduceScatter of projection partials that writes the output
tensor directly (each core keeps its 512 output columns).

Per q-chunk c: each core computes y[c] for its 8 heads, immediately
projects it against its 512 rows of w_proj into partials for ALL 1024
output columns, then a pair ReduceScatter sums the two cores' partials
and scatters column-halves. No AllGather of y, no separate proj phase.
"""
import numpy as np

D_MODEL = 1024
N_HEAD = 16
D_HEAD = 64
B = 4
T = 2048
N_CORES = 8
P = 128
PAIRS = 4          # head pairs per core
KT = D_MODEL // P  # 8 contraction tiles
QC = 512           # q chunk width
NQ = T // QC       # 4 q-chunks
# (q0, width) attention chunks; trailing chunks are narrower so the last
# ReduceScatter (serial tail) is small
CHUNKS = [(0, 512), (512, 512), (1024, 512), (1536, 256), (1792, 256)]

_RUNNER_CACHE = {}


def _build(has_qk_bias: bool):
    from concourse import bacc
    import concourse.mybir as mybir
    from concourse.tile import TileContext
    from concourse.bass import ts

    f32 = mybir.dt.float32
    f16 = mybir.dt.float16
    KD = D_MODEL + (1 if has_qk_bias else 0)

    nc = bacc.Bacc("TRN2", target_bir_lowering=False, debug=False,
                   num_devices=N_CORES)
    xT = nc.dram_tensor("xT", [KD, T], f16, kind="ExternalInput")
    wqk = nc.dram_tensor("wqk", [KD, 1024], f16, kind="ExternalInput")
    wv = nc.dram_tensor("wv", [D_MODEL, 512], f16, kind="ExternalInput")
    wp = nc.dram_tensor("wp", [512, 1024], f16, kind="ExternalInput")
    tri = nc.dram_tensor("tri", [P, 896], f16, kind="ExternalInput")
    out = nc.dram_tensor("out", [T, 512], f16, kind="ExternalOutput")

    with TileContext(nc) as tc:
        with (
            tc.tile_pool(name="xres", bufs=1) as x_res,
            tc.tile_pool(name="wres", bufs=1) as w_res,
            tc.tile_pool(name="qk_res", bufs=1) as qk_res,
            tc.tile_pool(name="v_res", bufs=1) as v_res,
            tc.tile_pool(name="dram", bufs=1, space="DRAM") as dram_pool,
        ):
            # ---- resident SBUF tensors -------------------------------------
            x_sb = x_res.tile([P, KT, T], f16, name="x_sb")
            wqk_sb = w_res.tile([P, KT, 1024], f16, name="wqk_sb")
            wv_sb = w_res.tile([P, KT, 512], f16, name="wv_sb")
            wp_sb = w_res.tile([P, PAIRS, 1024], f16, name="wp_sb")
            tri_sb = w_res.tile([P, 896], f16, name="tri_sb")
            qT = [qk_res.tile([P, T], f16, name=f"qT{p}") for p in range(PAIRS)]
            kT = [qk_res.tile([P, T], f16, name=f"kT{p}") for p in range(PAIRS)]
            v_sb = [v_res.tile([P, 8, 65], f16, name=f"v{t}")
                    for t in range(T // P)]
            if has_qk_bias:
                xrow = w_res.tile([1, T], f16, name="xrow")
                wrow = w_res.tile([1, 1024], f16, name="wrow")
                nc.sync.dma_start(out=xrow, in_=xT[D_MODEL:D_MODEL + 1, :])
                nc.sync.dma_start(out=wrow, in_=wqk[D_MODEL:D_MODEL + 1, :])

            nc.sync.dma_start(out=tri_sb, in_=tri[:])
            # interleave x / wqk loads per k-tile so the qk-proj accumulation
            # can start as soon as k-tile 0 lands; only pair-0's wqk columns
            # are needed up front
            nc.sync.dma_start(out=wqk_sb[:, 0, 0:256],
                              in_=wqk[0:P, 0:256])
            for n in range(NQ):
                nc.sync.dma_start(out=x_sb[:, 0, ts(n, QC)],
                                  in_=xT[0:P, ts(n, QC)])
            for k in range(1, KT):
                nc.sync.dma_start(out=x_sb[:, k, :], in_=xT[ts(k, P), :])
                nc.sync.dma_start(out=wqk_sb[:, k, 0:256],
                                  in_=wqk[ts(k, P), 0:256])
            for k in range(KT):
                nc.sync.dma_start(out=wqk_sb[:, k, 256:1024],
                                  in_=wqk[ts(k, P), 256:1024])
            for k in range(KT):
                nc.sync.dma_start(out=wv_sb[:, k, :], in_=wv[ts(k, P), :])
            for blk in range(PAIRS):
                nc.sync.dma_start(out=wp_sb[:, blk, :], in_=wp[ts(blk, P), :])

            # each chunk's RS input is [2, t-tiles, 128, 512]
            # (slot, t-tile, partition, out-col) — slot g holds this core's
            # partial for output columns [g*512, (g+1)*512)
            ag_in = [dram_pool.tile([2, w // P, P, 512], f16, name=f"ag_in{c}")
                     for c, (q0, w) in enumerate(CHUNKS)]
            rs_out = [dram_pool.tile([w, 512], f16, name=f"rs_out{c}")
                      for c, (q0, w) in enumerate(CHUNKS)]

            # ---- phase A: q/k projection for pairs 0-1 only ----------------
            # (pairs 2-3 are projected as PE filler inside attention chunk 0,
            # so the Activation engine starts on exp work ~30us earlier)
            with tc.tile_pool(name="psA", bufs=8, space="PSUM") as psA:
                # p-state warmup: zero-fed matmuls start the Tensor engine
                # ramping (2x slow until 3us continuously busy) while the
                # first x/wqk DMAs are still in flight
                wz = w_res.tile([P, 512], f16, name="wz")
                nc.vector.memset(wz[:], 0.0)
                wps = psA.tile([P, QC], f32, name="psA")
                for i in range(4):
                    nc.tensor.matmul(wps[:], wz[:, 0:P], wz[:],
                                     start=(i == 0), stop=(i == 3))
                slots = [(m, n) for m in range(2) for n in range(NQ)]
                acc = {}
                for mn in slots:
                    acc[mn] = psA.tile([P, QC], f32, name="psA")
                for k in range(KT):
                    for m, n in slots:
                        nc.tensor.matmul(
                            acc[(m, n)][:],
                            wqk_sb[:, k, m * P:(m + 1) * P],
                            x_sb[:, k, ts(n, QC)],
                            start=(k == 0),
                            stop=(k == KT - 1) and not has_qk_bias)
                if has_qk_bias:
                    for m, n in slots:
                        nc.tensor.matmul(
                            acc[(m, n)][:],
                            wrow[:, m * P:(m + 1) * P],
                            xrow[:, ts(n, QC)],
                            start=False, stop=True)
                for m, n in slots:
                    dest = qT[0] if m == 0 else kT[0]
                    nc.vector.tensor_copy(out=dest[:, ts(n, QC)],
                                          in_=acc[(m, n)][:])

                for m in range(2):
                    for n in range(NQ):
                        ps = psA.tile([P, QC], f32, name="psA")
                        for k in range(KT):
                            nc.tensor.matmul(
                                ps[:],
                                wqk_sb[:, k, 256 + m * P:256 + (m + 1) * P],
                                x_sb[:, k, ts(n, QC)],
                                start=(k == 0),
                                stop=(k == KT - 1) and not has_qk_bias)
                        if has_qk_bias:
                            nc.tensor.matmul(
                                ps[:],
                                wrow[:, 256 + m * P:256 + (m + 1) * P],
                                xrow[:, ts(n, QC)],
                                start=False, stop=True)
                        dest = qT[1] if m == 0 else kT[1]
                        nc.vector.tensor_copy(out=dest[:, ts(n, QC)],
                                              in_=ps[:])

            # ---- phase B: chunk-major attention + partial proj + RS --------
            # Chunks are processed out of q-order: the three 512-wide chunks
            # first (Act-efficient big exps, st bufs=3), then the two 256-wide
            # chunks in a second pool scope (1-bank st tiles, bufs=4 => deeper
            # QK->exp pipeline). The diag-only (0,256) chunk runs LAST so the
            # serial tail (its attention + pp + ReduceScatter) is minimal.
            # PE "filler" units with always-ready deps (V tiles, pair-2/3
            # q/k projection, previous chunk's partial proj) are spread
            # between attention slots so exp backlog never dries up.
            with (
                tc.tile_pool(name="ex", bufs=7) as ex_pool,
                tc.tile_pool(name="yc", bufs=2) as yc_pool,
                tc.tile_pool(name="pp", bufs=2) as ppsb_pool,
                tc.tile_pool(name="rr", bufs=4) as r_pool,
                tc.tile_pool(name="rb", bufs=4) as rb_pool,
            ):
                state = {"prev": None, "pending": [], "vdone": 0,
                         "rs_ready": []}

                def emit_rs():
                    while state["rs_ready"]:
                        ci = state["rs_ready"].pop(0)
                        cq0, cw = CHUNKS[ci]
                        nc.gpsimd.collective_compute(
                            "ReduceScatter",
                            mybir.AluOpType.add,
                            ins=[ag_in[ci][:].opt()],
                            outs=[rs_out[ci][:].opt()],
                            replica_groups=[[0, 1], [2, 3], [4, 5], [6, 7]],
                        )

                def flush_pending():
                    pending = state["pending"]
                    while pending:
                        y_ps_, y_c_, pb_, p_, w_ = pending.pop(0)
                        r = r_pool.tile([1, 512], f32, name="rden", tag="r")
                        nc.vector.reciprocal(out=r[:, 0:w_],
                                             in_=y_ps_[64:65, 0:w_])
                        rb = rb_pool.tile([64, 512], f32, name="rb", tag="rb")
                        nc.gpsimd.partition_broadcast(rb[:, 0:w_], r[:, 0:w_])
                        nc.vector.tensor_mul(
                            y_c_[pb_:pb_ + 64, p_, 0:w_],
                            y_ps_[0:64, 0:w_], rb[:, 0:w_])

                def make_units(acc_ps):
                    def v_tile(tt):
                        ps = acc_ps("psV")
                        for k in range(KT):
                            nc.tensor.matmul(ps[:],
                                             x_sb[:, k, ts(tt, P)],
                                             wv_sb[:, k, :],
                                             start=(k == 0),
                                             stop=(k == KT - 1))
                        nc.vector.memset(v_sb[tt][:, :, 64:65], 1.0)
                        src = ps.rearrange("p (h c) -> p h c", c=64)
                        nc.vector.tensor_copy(out=v_sb[tt][:, :, 0:64],
                                              in_=src[:])

                    def qk_unit(p, m, n):
                        ps = acc_ps("psQK")
                        for k in range(KT):
                            nc.tensor.matmul(
                                ps[:],
                                wqk_sb[:, k,
                                       p * 256 + m * P:p * 256 + (m + 1) * P],
                                x_sb[:, k, ts(n, QC)],
                                start=(k == 0),
                                stop=(k == KT - 1) and not has_qk_bias)
                        if has_qk_bias:
                            nc.tensor.matmul(
                                ps[:],
                                wrow[:, p * 256 + m * P:p * 256 + (m + 1) * P],
                                xrow[:, ts(n, QC)],
                                start=False, stop=True)
                        dest = qT[p] if m == 0 else kT[p]
                        nc.vector.tensor_copy(out=dest[:, ts(n, QC)],
                                              in_=ps[:])

                    def pp_group(ci, y_c, pp_sb, tt, hf):
                        q0, w = CHUNKS[ci]
                        nt = w // P
                        ps = acc_ps("ppps")
                        for p in range(PAIRS):
                            nc.tensor.matmul(
                                ps[:],
                                y_c[:, p, ts(tt, P)],
                                wp_sb[:, p, hf * 512:(hf + 1) * 512],
                                start=(p == 0), stop=(p == PAIRS - 1))
                        nc.vector.tensor_copy(
                            out=pp_sb[:, tt, hf * 512:(hf + 1) * 512],
                            in_=ps[:])
                        if tt == nt - 1 and hf == 1:
                            # last partial of chunk ci: ship it; the RS
                            # dispatch is deferred until its DMA-done wait
                            # is already satisfied (else it head-blocks the
                            # Pool sequencer and stalls normalize bcasts)
                            for f in range(2):
                                nc.sync.dma_start(
                                    out=ag_in[ci][f].rearrange(
                                        "t p o -> p t o"),
                                    in_=pp_sb[:, 0:nt,
                                              f * 512:(f + 1) * 512])
                            state["rs_ready"].append(ci)

                    return v_tile, qk_unit, pp_group

                def chunk_groups(q0, w):
                    nsub = q0 // P
                    nd = w // P
                    groups = [[(2 * g, 0, 0, w, 0, False),
                               (2 * g + 1, 1, 0, w, 0, False)]
                              for g in range(nsub // 2)]
                    d = nsub
                    if nd == 4:
                        groups.append([(d + 0, 0, 0, 512, 0, True),
                                       (d + 1, 1, 0, 384, 128, True),
                                       (d + 3, 1, 384, 512, 384, True)])
                        groups.append([(d + 2, 0, 256, 512, 256, True)])
                    else:  # nd == 2: two rows, ragged -> per-row exps
                        groups.append([(d + 0, 0, 0, 256, 0, True),
                                       (d + 1, 1, 0, 128, 128, True)])
                    return groups

                def process_chunk(ci, st_pool, stw, units, qk_fill):
                    v_tile, qk_unit, pp_group = units
                    q0, w = CHUNKS[ci]
                    nt = w // P
                    y_c = yc_pool.tile([P, PAIRS, QC], f16, name="yc")
                    flush_after = [[] for _ in range(8)]
                    if state["prev"] is not None:
                        pci, py_c, ppp_sb = state["prev"]
                        pnt = CHUNKS[pci][1] // P
                        items = [(pp_group, (pci, py_c, ppp_sb, tt, hf))
                                 for tt in range(pnt) for hf in range(2)]
                        # spread over slots 0-2: the last unit (which ships
                        # the ag DMAs) still lands before emit_rs at slot 3
                        for i, it in enumerate(items):
                            flush_after[i * 5 // len(items)].append(it)
                    for i, it in enumerate(qk_fill):
                        flush_after[i * 4 // len(qk_fill)].append(it)
                    if ci + 1 < len(CHUNKS):
                        nq0, nw = CHUNKS[ci + 1]
                        need = (nq0 + nw) // P
                        vt = [(v_tile, (tt,))
                              for tt in range(state["vdone"], need)]
                        state["vdone"] = max(state["vdone"], need)
                        for i, it in enumerate(vt):
                            flush_after[3 + (i * 5) // max(len(vt), 1)
                                        ].append(it)

                    groups = chunk_groups(q0, w)
                    n_mm = sum(len(g) for g in groups)
                    n_grp = len(groups)
                    slot_ys = {}

                    def emit_front(si, gi, grp):
                        p, h = si // 2, si % 2
                        pb = h * 64
                        st = st_pool.tile([P, 2, stw], f32, name="st",
                                          tag="st")
                        ex = ex_pool.tile([P, 2, 512], f16, name="ex")
                        for kt, row, dlo, dhi, qlo, _ in grp:
                            nc.tensor.matmul(
                                st[:, row, dlo:dhi],
                                kT[p][pb:pb + 64, ts(kt, P)],
                                qT[p][pb:pb + 64,
                                      q0 + qlo:q0 + qlo + dhi - dlo],
                                start=True, stop=True)
                        spans = {}
                        for e in grp:
                            lo, hi = spans.get(e[1], (e[2], e[3]))
                            spans[e[1]] = (min(lo, e[2]), max(hi, e[3]))
                        if (len(spans) == 2
                                and len(set(spans.values())) == 1):
                            lo, hi = spans[0]
                            nc.scalar.activation(
                                ex[:, :, lo:hi], st[:, :, lo:hi],
                                mybir.ActivationFunctionType.Exp,
                                scale=0.125)
                        else:
                            for row, (lo, hi) in spans.items():
                                nc.scalar.activation(
                                    ex[:, row, lo:hi], st[:, row, lo:hi],
                                    mybir.ActivationFunctionType.Exp,
                                    scale=0.125)
                        for kt, row, dlo, dhi, qlo, masked in grp:
                            if masked:  # intra-tile causal triangle
                                nc.vector.tensor_mul(
                                    ex[:, row, dlo:dhi],
                                    ex[:, row, dlo:dhi],
                                    tri_sb[:, 384:384 + dhi - dlo])
                        if gi == 0:
                            flush_pending()
                            if si == 7:
                                emit_rs()
                        return ex

                    def emit_pv(si, gi, grp, ex):
                        p, h = si // 2, si % 2
                        pb = h * 64
                        lh = 2 * p + h
                        if si not in slot_ys:
                            slot_ys[si] = (y_pool.tile([P, 512], f32,
                                                       name="yps"), [0])
                        y_ps, cnt = slot_ys[si]
                        for kt, row, dlo, dhi, qlo, masked in grp:
                            nc.tensor.matmul(
                                y_ps[0:65, qlo:qlo + dhi - dlo],
                                v_sb[kt][:, lh, :],
                                ex[:, row, dlo:dhi],
                                start=(cnt[0] == 0),
                                stop=(cnt[0] == n_mm - 1))
                            cnt[0] += 1
                        if gi == n_grp - 1:
                            state["pending"].append((y_ps, y_c, pb, p, w))
                            for fn, args in flush_after[si]:
                                fn(*args)

                    # software-pipelined: group g+1's QK before group g's PV
                    units_l = [(si, gi, grp)
                               for si in range(8)
                               for gi, grp in enumerate(groups)]
                    inflight = []
                    for u in units_l:
                        ex = emit_front(*u)
                        if len(inflight) >= 3:
                            pu, pex = inflight.pop(0)
                            emit_pv(*pu, ex=pex)
                        inflight.append((u, ex))
                    for pu, pex in inflight:
                        emit_pv(*pu, ex=pex)
                    flush_pending()
                    state["prev"] = (ci, y_c,
                                     ppsb_pool.tile([P, 4, 1024], f16,
                                                    name="ppsb"))

                # ---- B1: the 512-wide chunks, st bufs=3 --------------------
                with (
                    tc.tile_pool(name="st1", bufs=3, space="PSUM") as st1,
                    tc.tile_pool(name="yp1", bufs=2, space="PSUM") as y_pool,
                ):
                    def acc1(name):
                        t = st1.tile([P, 2, 512], f32, name=name, tag="st")
                        return t[:, 0, :]

                    units1 = make_units(acc1)
                    v_tile1 = units1[0]
                    qk_unit1 = units1[1]
                    for tt in range(4):
                        v_tile1(tt)
                    state["vdone"] = 4
                    qk_fill = [(qk_unit1, (2, m, n))
                               for m in range(2) for n in range(NQ)]
                    qk_fill += [(qk_unit1, (3, m, n))
                                for m in range(2) for n in range(NQ)]
                    process_chunk(0, st1, 512, units1, qk_fill)
                    process_chunk(1, st1, 512, units1, [])
                    process_chunk(2, st1, 512, units1, [])

                # ---- B2: the 256-wide chunks, 1-bank st tiles, bufs=4 ------
                with (
                    tc.tile_pool(name="st2", bufs=6, space="PSUM") as st2,
                    tc.tile_pool(name="yp2", bufs=2, space="PSUM") as y_pool,
                ):
                    def acc2(name):
                        t = st2.tile([P, 2, 256], f32, name=name, tag="st")
                        return t.rearrange("p a b -> p (a b)")

                    units2 = make_units(acc2)
                    process_chunk(3, st2, 256, units2, [])
                    process_chunk(4, st2, 256, units2, [])

                    # final chunk's partial projection (tail — kept small)
                    pci, py_c, ppp_sb = state["prev"]
                    for tt in range(CHUNKS[pci][1] // P):
                        for hf in range(2):
                            units2[2](pci, py_c, ppp_sb, tt, hf)
                    emit_rs()
                    for ci, (cq0, cw) in enumerate(CHUNKS):
                        nc.sync.dma_start(out=out[cq0:cq0 + cw, :],
                                          in_=rs_out[ci][:])
    nc.compile()
    return nc


def _make_runner(nc):
    """Reusable 8-core SPMD runner (jit built once)."""
    import jax
    from jax.sharding import Mesh, PartitionSpec
    from jax.experimental.shard_map import shard_map
    from concourse import bass2jax
    import concourse.mybir as mybir

    bass2jax.install_neuronx_cc_hook()
    partition_name = (nc.partition_id_tensor.name
                      if nc.partition_id_tensor else None)
    in_names, out_names, out_avals, zero_outs = [], [], [], []
    for alloc in nc.m.functions[0].allocations:
        if not isinstance(alloc, mybir.MemoryLocationSet):
            continue
        name = alloc.memorylocations[0].name
        if alloc.kind == "ExternalInput":
            if name != partition_name:
                in_names.append(name)
        elif alloc.kind == "ExternalOutput":
            shape = tuple(alloc.tensor_shape)
            dtype = mybir.dt.np(alloc.dtype)
            out_names.append(name)
            out_avals.append(jax.core.ShapedArray(shape, dtype))
            zero_outs.append(np.zeros(shape, dtype))
    n_params = len(in_names)
    n_outs = len(out_avals)
    all_in = list(in_names) + list(out_names)
    if partition_name is not None:
        all_in.append(partition_name)

    def _body(*args):
        operands = list(args)
        if partition_name is not None:
            operands.append(bass2jax.partition_id_tensor())
        outs = bass2jax._bass_exec_p.bind(
            *operands,
            out_avals=tuple(out_avals),
            in_names=tuple(all_in),
            out_names=tuple(out_names),
            lowering_input_output_aliases=(),
            sim_require_finite=True,
            sim_require_nnan=True,
            nc=nc,
        )
        return tuple(outs)

    devices = jax.devices()[:N_CORES]
    mesh = Mesh(np.asarray(devices), ("core",))
    in_specs = (PartitionSpec("core"),) * (n_params + n_outs)
    out_specs = (PartitionSpec("core"),) * n_outs
    donate = tuple(range(n_params, n_params + n_outs))
    sharded = jax.jit(
        shard_map(_body, mesh=mesh, in_specs=in_specs, out_specs=out_specs,
                  check_rep=False),
        donate_argnums=donate, keep_unused=True)

    def run(in_maps):
        per_core = [[np.asarray(m[k]) for k in in_names] for m in in_maps]
        concat_in = [
            np.concatenate([per_core[c][i] for c in range(N_CORES)], axis=0)
            for i in range(n_params)]
        concat_zeros = [
            np.zeros((N_CORES * z.shape[0], *z.shape[1:]), z.dtype)
            for z in zero_outs]
        outs = sharded(*concat_in, *concat_zeros)
        jax.block_until_ready(outs)
        return [
            {name: np.asarray(outs[i]).reshape(N_CORES, *out_avals[i].shape)[c]
             for i, name in enumerate(out_names)}
            for c in range(N_CORES)]

    return run


def kernel(x, w_qkv, b_qkv, w_proj, b_proj):
    x = np.asarray(x, dtype=np.float32)
    w_qkv = np.asarray(w_qkv, dtype=np.float32)
    b_qkv = np.asarray(b_qkv, dtype=np.float32)
    w_proj = np.asarray(w_proj, dtype=np.float32)
    b_proj = np.asarray(b_proj, dtype=np.float32)

    w_q, w_k, w_v = w_qkv[0:1024], w_qkv[1024:2048], w_qkv[2048:3072]
    b_q, b_k, b_v = b_qkv[0:1024], b_qkv[1024:2048], b_qkv[2048:3072]
    has_qk_bias = bool(np.any(b_q) or np.any(b_k))

    key = ("runner", has_qk_bias)
    if key not in _RUNNER_CACHE:
        nc = _build(has_qk_bias)
        _RUNNER_CACHE[key] = _make_runner(nc)
    run = _RUNNER_CACHE[key]

    # causal mask lookup: tri[k, m] = 1.0 iff k <= m - 384
    kk = np.arange(P)[:, None]
    mm = np.arange(896)[None, :]
    tri = (kk <= mm - 384).astype(np.float16)

    in_maps = []
    for core in range(N_CORES):
        b, g = divmod(core, 2)
        xT_c = np.ascontiguousarray(x[b].T).astype(np.float16)  # [1024, 2048]
        if has_qk_bias:
            xT_c = np.concatenate(
                [xT_c, np.ones((1, T), np.float16)], axis=0)
        wqk_c = np.empty((D_MODEL + (1 if has_qk_bias else 0), 1024),
                         np.float16)
        for p in range(PAIRS):
            hA = 8 * g + 2 * p
            hB = hA + 1
            cols = p * 256
            wqk_c[:D_MODEL, cols + 0:cols + 64] = w_q[hA * 64:(hA + 1) * 64].T
            wqk_c[:D_MODEL, cols + 64:cols + 128] = w_q[hB * 64:(hB + 1) * 64].T
            wqk_c[:D_MODEL, cols + 128:cols + 192] = w_k[hA * 64:(hA + 1) * 64].T
            wqk_c[:D_MODEL, cols + 192:cols + 256] = w_k[hB * 64:(hB + 1) * 64].T
            if has_qk_bias:
                wqk_c[D_MODEL, cols + 0:cols + 64] = b_q[hA * 64:(hA + 1) * 64]
                wqk_c[D_MODEL, cols + 64:cols + 128] = b_q[hB * 64:(hB + 1) * 64]
                wqk_c[D_MODEL, cols + 128:cols + 192] = b_k[hA * 64:(hA + 1) * 64]
                wqk_c[D_MODEL, cols + 192:cols + 256] = b_k[hB * 64:(hB + 1) * 64]
        wv_c = np.ascontiguousarray(
            w_v[8 * g * 64:(8 * g + 8) * 64].T).astype(np.float16)
        # wp rows = this core's 512 y-columns (its 8 heads), all 1024 outputs
        wp_c = np.ascontiguousarray(
            w_proj.T[g * 512:(g + 1) * 512, :]).astype(np.float16)
        in_maps.append({
            "xT": xT_c, "wqk": wqk_c, "wv": wv_c, "wp": wp_c, "tri": tri,
        })

    results = run(in_maps)

    out = np.empty((B, T, D_MODEL), dtype=np.float32)
    for core in range(N_CORES):
        b, g = divmod(core, 2)
        out[b, :, g * 512:(g + 1) * 512] = results[core]["out"].astype(
            np.float32)

    # exact host-side bias folds (v-bias rides softmax row-sums == 1;
    # proj bias is additive)
    if np.any(b_v):
        out += (b_v @ w_proj.T)[None, None, :]
    if np.any(b_proj):
        out += b_proj[None, None, :]
    return out


# revision 8
# speedup vs baseline: 1.0068x; 1.0068x over previous
"""Causal self-attention (B=4, T=2048, C=1024, H=16) on 8 TRN2 NeuronCores.

Sharding: core = 2*b + g  (b = batch 0..3, g = head-group of 8 heads).
v2: full f16 datapath (f32 PSUM accumulation), x resident in SBUF,
chunk-major attention, per-chunk partial output projection, and a
pairwise ReduceScatter of projection partials that writes the output
tensor directly (each core keeps its 512 output columns).

Per q-chunk c: each core computes y[c] for its 8 heads, immediately
projects it against its 512 rows of w_proj into partials for ALL 1024
output columns, then a pair ReduceScatter sums the two cores' partials
and scatters column-halves. No AllGather of y, no separate proj phase.
"""
import numpy as np

D_MODEL = 1024
N_HEAD = 16
D_HEAD = 64
B = 4
T = 2048
N_CORES = 8
P = 128
PAIRS = 4          # head pairs per core
KT = D_MODEL // P  # 8 contraction tiles
QC = 512           # q chunk width
NQ = T // QC       # 4 q-chunks
# (q0, width) attention chunks; trailing chunks are narrower so the last
# ReduceScatter (serial tail) is small
CHUNKS = [(0, 512), (512, 512), (1024, 512), (1536, 256), (1792, 256)]

_RUNNER_CACHE = {}


def _build(has_qk_bias: bool):
    from concourse import bacc
    import concourse.mybir as mybir
    from concourse.tile import TileContext
    from concourse.bass import ts

    f32 = mybir.dt.float32
    f16 = mybir.dt.float16
    KD = D_MODEL + (1 if has_qk_bias else 0)

    nc = bacc.Bacc("TRN2", target_bir_lowering=False, debug=False,
                   num_devices=N_CORES)
    xT = nc.dram_tensor("xT", [KD, T], f16, kind="ExternalInput")
    wqk = nc.dram_tensor("wqk", [KD, 1024], f16, kind="ExternalInput")
    wv = nc.dram_tensor("wv", [D_MODEL, 512], f16, kind="ExternalInput")
    wp = nc.dram_tensor("wp", [512, 1024], f16, kind="ExternalInput")
    tri = nc.dram_tensor("tri", [P, 896], f16, kind="ExternalInput")
    out = nc.dram_tensor("out", [T, 512], f16, kind="ExternalOutput")

    with TileContext(nc) as tc:
        with (
            tc.tile_pool(name="xres", bufs=1) as x_res,
            tc.tile_pool(name="wres", bufs=1) as w_res,
            tc.tile_pool(name="qk_res", bufs=1) as qk_res,
            tc.tile_pool(name="v_res", bufs=1) as v_res,
            tc.tile_pool(name="dram", bufs=1, space="DRAM") as dram_pool,
        ):
            # ---- resident SBUF tensors -------------------------------------
            x_sb = x_res.tile([P, KT, T], f16, name="x_sb")
            wqk_sb = w_res.tile([P, KT, 1024], f16, name="wqk_sb")
            wv_sb = w_res.tile([P, KT, 512], f16, name="wv_sb")
            wp_sb = w_res.tile([P, PAIRS, 1024], f16, name="wp_sb")
            tri_sb = w_res.tile([P, 896], f16, name="tri_sb")
            qT = [qk_res.tile([P, T], f16, name=f"qT{p}") for p in range(PAIRS)]
            kT = [qk_res.tile([P, T], f16, name=f"kT{p}") for p in range(PAIRS)]
            v_sb = [v_res.tile([P, 8, 65], f16, name=f"v{t}")
                    for t in range(T // P)]
            if has_qk_bias:
                xrow = w_res.tile([1, T], f16, name="xrow")
                wrow = w_res.tile([1, 1024], f16, name="wrow")
                nc.sync.dma_start(out=xrow, in_=xT[D_MODEL:D_MODEL + 1, :])
                nc.sync.dma_start(out=wrow, in_=wqk[D_MODEL:D_MODEL + 1, :])

            nc.sync.dma_start(out=tri_sb, in_=tri[:])
            # interleave x / wqk loads per k-tile so the qk-proj accumulation
            # can start as soon as k-tile 0 lands; only pair-0's wqk columns
            # are needed up front
            nc.sync.dma_start(out=wqk_sb[:, 0, 0:256],
                              in_=wqk[0:P, 0:256])
            for n in range(NQ):
                nc.sync.dma_start(out=x_sb[:, 0, ts(n, QC)],
                                  in_=xT[0:P, ts(n, QC)])
            for k in range(1, KT):
                nc.sync.dma_start(out=x_sb[:, k, :], in_=xT[ts(k, P), :])
                nc.sync.dma_start(out=wqk_sb[:, k, 0:256],
                                  in_=wqk[ts(k, P), 0:256])
            for k in range(KT):
                nc.sync.dma_start(out=wqk_sb[:, k, 256:1024],
                                  in_=wqk[ts(k, P), 256:1024])
            for k in range(KT):
                nc.sync.dma_start(out=wv_sb[:, k, :], in_=wv[ts(k, P), :])
            for blk in range(PAIRS):
                nc.sync.dma_start(out=wp_sb[:, blk, :], in_=wp[ts(blk, P), :])

            # each chunk's RS input is [2, t-tiles, 128, 512]
            # (slot, t-tile, partition, out-col) — slot g holds this core's
            # partial for output columns [g*512, (g+1)*512)
            ag_in = [dram_pool.tile([2, w // P, P, 512], f16, name=f"ag_in{c}")
                     for c, (q0, w) in enumerate(CHUNKS)]
            rs_out = [dram_pool.tile([w, 512], f16, name=f"rs_out{c}")
                      for c, (q0, w) in enumerate(CHUNKS)]

            # ---- phase A: q/k projection for pairs 0-1 only ----------------
            # (pairs 2-3 are projected as PE filler inside attention chunk 0,
            # so the Activation engine starts on exp work ~30us earlier)
            with tc.tile_pool(name="psA", bufs=8, space="PSUM") as psA:
                # p-state warmup: zero-fed matmuls start the Tensor engine
                # ramping (2x slow until 3us continuously busy) while the
                # first x/wqk DMAs are still in flight
                wz = w_res.tile([P, 512], f16, name="wz")
                nc.vector.memset(wz[:], 0.0)
                wps = psA.tile([P, QC], f32, name="psA")
                for i in range(4):
                    nc.tensor.matmul(wps[:], wz[:, 0:P], wz[:],
                                     start=(i == 0), stop=(i == 3))
                slots = [(m, n) for m in range(2) for n in range(NQ)]
                acc = {}
                for mn in slots:
                    acc[mn] = psA.tile([P, QC], f32, name="psA")
                for k in range(KT):
                    for m, n in slots:
                        nc.tensor.matmul(
                            acc[(m, n)][:],
                            wqk_sb[:, k, m * P:(m + 1) * P],
                            x_sb[:, k, ts(n, QC)],
                            start=(k == 0),
                            stop=(k == KT - 1) and not has_qk_bias)
                if has_qk_bias:
                    for m, n in slots:
                        nc.tensor.matmul(
                            acc[(m, n)][:],
                            wrow[:, m * P:(m + 1) * P],
                            xrow[:, ts(n, QC)],
                            start=False, stop=True)
                for m, n in slots:
                    dest = qT[0] if m == 0 else kT[0]
                    nc.vector.tensor_copy(out=dest[:, ts(n, QC)],
                                          in_=acc[(m, n)][:])

                for m in range(2):
                    for n in range(NQ):
                        ps = psA.tile([P, QC], f32, name="psA")
                        for k in range(KT):
                            nc.tensor.matmul(
                                ps[:],
                                wqk_sb[:, k, 256 + m * P:256 + (m + 1) * P],
                                x_sb[:, k, ts(n, QC)],
                                start=(k == 0),
                                stop=(k == KT - 1) and not has_qk_bias)
                        if has_qk_bias:
                            nc.tensor.matmul(
                                ps[:],
                                wrow[:, 256 + m * P:256 + (m + 1) * P],
                                xrow[:, ts(n, QC)],
                                start=False, stop=True)
                        dest = qT[1] if m == 0 else kT[1]
                        nc.vector.tensor_copy(out=dest[:, ts(n, QC)],
                                              in_=ps[:])

            # ---- phase B: chunk-major attention + partial proj + RS --------
            # Chunks are processed out of q-order: the three 512-wide chunks
            # first (Act-efficient big exps, st bufs=3), then the two 256-wide
            # chunks in a second pool scope (1-bank st tiles, bufs=4 => deeper
            # QK->exp pipeline). The diag-only (0,256) chunk runs LAST so the
            # serial tail (its attention + pp + ReduceScatter) is minimal.
            # PE "filler" units with always-ready deps (V tiles, pair-2/3
            # q/k projection, previous chunk's partial proj) are spread
            # between attention slots so exp backlog never dries up.
            with (
                tc.tile_pool(name="ex", bufs=7) as ex_pool,
                tc.tile_pool(name="yc", bufs=2) as yc_pool,
                tc.tile_pool(name="pp", bufs=2) as ppsb_pool,
                tc.tile_pool(name="rr", bufs=4) as r_pool,
                tc.tile_pool(name="rb", bufs=4) as rb_pool,
            ):
                state = {"prev": None, "pending": [], "vdone": 0,
                         "rs_ready": []}

                def emit_rs():
                    while state["rs_ready"]:
                        ci = state["rs_ready"].pop(0)
                        cq0, cw = CHUNKS[ci]
                        nc.gpsimd.collective_compute(
                            "ReduceScatter",
                            mybir.AluOpType.add,
                            ins=[ag_in[ci][:].opt()],
                            outs=[rs_out[ci][:].opt()],
                            replica_groups=[[0, 1], [2, 3], [4, 5], [6, 7]],
                        )

                def flush_pending():
                    pending = state["pending"]
                    while pending:
                        y_ps_, y_c_, pb_, p_, w_ = pending.pop(0)
                        r = r_pool.tile([1, 512], f32, name="rden", tag="r")
                        nc.vector.reciprocal(out=r[:, 0:w_],
                                             in_=y_ps_[64:65, 0:w_])
                        rb = rb_pool.tile([64, 512], f32, name="rb", tag="rb")
                        nc.gpsimd.partition_broadcast(rb[:, 0:w_], r[:, 0:w_])
                        nc.vector.tensor_mul(
                            y_c_[pb_:pb_ + 64, p_, 0:w_],
                            y_ps_[0:64, 0:w_], rb[:, 0:w_])

                def make_units(acc_ps):
                    def v_tile(tt):
                        ps = acc_ps("psV")
                        for k in range(KT):
                            nc.tensor.matmul(ps[:],
                                             x_sb[:, k, ts(tt, P)],
                                             wv_sb[:, k, :],
                                             start=(k == 0),
                                             stop=(k == KT - 1))
                        nc.vector.memset(v_sb[tt][:, :, 64:65], 1.0)
                        src = ps.rearrange("p (h c) -> p h c", c=64)
                        nc.vector.tensor_copy(out=v_sb[tt][:, :, 0:64],
                                              in_=src[:])

                    def qk_unit(p, m, n):
                        ps = acc_ps("psQK")
                        for k in range(KT):
                            nc.tensor.matmul(
                                ps[:],
                                wqk_sb[:, k,
                                       p * 256 + m * P:p * 256 + (m + 1) * P],
                                x_sb[:, k, ts(n, QC)],
                                start=(k == 0),
                                stop=(k == KT - 1) and not has_qk_bias)
                        if has_qk_bias:
                            nc.tensor.matmul(
                                ps[:],
                                wrow[:, p * 256 + m * P:p * 256 + (m + 1) * P],
                                xrow[:, ts(n, QC)],
                                start=False, stop=True)
                        dest = qT[p] if m == 0 else kT[p]
                        nc.vector.tensor_copy(out=dest[:, ts(n, QC)],
                                              in_=ps[:])

                    def pp_group(ci, y_c, pp_sb, tt, hf):
                        q0, w = CHUNKS[ci]
                        nt = w // P
                        ps = acc_ps("ppps")
                        for p in range(PAIRS):
                            nc.tensor.matmul(
                                ps[:],
                                y_c[:, p, ts(tt, P)],
                                wp_sb[:, p, hf * 512:(hf + 1) * 512],
                                start=(p == 0), stop=(p == PAIRS - 1))
                        nc.vector.tensor_copy(
                            out=pp_sb[:, tt, hf * 512:(hf + 1) * 512],
                            in_=ps[:])
                        if tt == nt - 1 and hf == 1:
                            # last partial of chunk ci: ship it; the RS
                            # dispatch is deferred until its DMA-done wait
                            # is already satisfied (else it head-blocks the
                            # Pool sequencer and stalls normalize bcasts)
                            for f in range(2):
                                nc.sync.dma_start(
                                    out=ag_in[ci][f].rearrange(
                                        "t p o -> p t o"),
                                    in_=pp_sb[:, 0:nt,
                                              f * 512:(f + 1) * 512])
                            state["rs_ready"].append(ci)

                    return v_tile, qk_unit, pp_group

                def chunk_groups(q0, w):
                    nsub = q0 // P
                    nd = w // P
                    groups = [[(2 * g, 0, 0, w, 0, False),
                               (2 * g + 1, 1, 0, w, 0, False)]
                              for g in range(nsub // 2)]
                    d = nsub
                    if nd == 4:
                        groups.append([(d + 0, 0, 0, 512, 0, True),
                                       (d + 1, 1, 0, 384, 128, True),
                                       (d + 3, 1, 384, 512, 384, True)])
                        groups.append([(d + 2, 0, 256, 512, 256, True)])
                    else:  # nd == 2: two rows, ragged -> per-row exps
                        groups.append([(d + 0, 0, 0, 256, 0, True),
                                       (d + 1, 1, 0, 128, 128, True)])
                    return groups

                def process_chunk(ci, st_pool, stw, units, qk_fill):
                    v_tile, qk_unit, pp_group = units
                    q0, w = CHUNKS[ci]
                    nt = w // P
                    y_c = yc_pool.tile([P, PAIRS, QC], f16, name="yc")
                    flush_after = [[] for _ in range(8)]
                    if state["prev"] is not None:
                        pci, py_c, ppp_sb = state["prev"]
                        pnt = CHUNKS[pci][1] // P
                        items = [(pp_group, (pci, py_c, ppp_sb, tt, hf))
                                 for tt in range(pnt) for hf in range(2)]
                        # spread over slots 0-2: the last unit (which ships
                        # the ag DMAs) still lands before emit_rs at slot 3
                        for i, it in enumerate(items):
                            flush_after[i * 5 // len(items)].append(it)
                    for i, it in enumerate(qk_fill):
                        flush_after[i * 4 // len(qk_fill)].append(it)
                    if ci + 1 < len(CHUNKS):
                        nq0, nw = CHUNKS[ci + 1]
                        need = (nq0 + nw) // P
                        vt = [(v_tile, (tt,))
                              for tt in range(state["vdone"], need)]
                        state["vdone"] = max(state["vdone"], need)
                        for i, it in enumerate(vt):
                            flush_after[5 + (i * 3) // max(len(vt), 1)
                                        ].append(it)

                    groups = chunk_groups(q0, w)
                    n_mm = sum(len(g) for g in groups)
                    n_grp = len(groups)
                    slot_ys = {}

                    def emit_front(si, gi, grp):
                        p, h = si // 2, si % 2
                        pb = h * 64
                        st = st_pool.tile([P, 2, stw], f32, name="st",
                                          tag="st")
                        ex = ex_pool.tile([P, 2, 512], f16, name="ex")
                        for kt, row, dlo, dhi, qlo, _ in grp:
                            nc.tensor.matmul(
                                st[:, row, dlo:dhi],
                                kT[p][pb:pb + 64, ts(kt, P)],
                                qT[p][pb:pb + 64,
                                      q0 + qlo:q0 + qlo + dhi - dlo],
                                start=True, stop=True)
                        spans = {}
                        for e in grp:
                            lo, hi = spans.get(e[1], (e[2], e[3]))
                            spans[e[1]] = (min(lo, e[2]), max(hi, e[3]))
                        if (len(spans) == 2
                                and len(set(spans.values())) == 1):
                            lo, hi = spans[0]
                            nc.scalar.activation(
                                ex[:, :, lo:hi], st[:, :, lo:hi],
                                mybir.ActivationFunctionType.Exp,
                                scale=0.125)
                        else:
                            for row, (lo, hi) in spans.items():
                                nc.scalar.activation(
                                    ex[:, row, lo:hi], st[:, row, lo:hi],
                                    mybir.ActivationFunctionType.Exp,
                                    scale=0.125)
                        for kt, row, dlo, dhi, qlo, masked in grp:
                            if masked:  # intra-tile causal triangle
                                nc.vector.tensor_mul(
                                    ex[:, row, dlo:dhi],
                                    ex[:, row, dlo:dhi],
                                    tri_sb[:, 384:384 + dhi - dlo])
                        if gi == 0:
                            flush_pending()
                            if si == 7:
                                emit_rs()
                        return ex

                    def emit_pv(si, gi, grp, ex):
                        p, h = si // 2, si % 2
                        pb = h * 64
                        lh = 2 * p + h
                        if si not in slot_ys:
                            slot_ys[si] = (y_pool.tile([P, 512], f32,
                                                       name="yps"), [0])
                        y_ps, cnt = slot_ys[si]
                        for kt, row, dlo, dhi, qlo, masked in grp:
                            nc.tensor.matmul(
                                y_ps[0:65, qlo:qlo + dhi - dlo],
                                v_sb[kt][:, lh, :],
                                ex[:, row, dlo:dhi],
                                start=(cnt[0] == 0),
                                stop=(cnt[0] == n_mm - 1))
                            cnt[0] += 1
                        if gi == n_grp - 1:
                            state["pending"].append((y_ps, y_c, pb, p, w))
                            for fn, args in flush_after[si]:
                                fn(*args)

                    # software-pipelined: group g+1's QK before group g's PV
                    units_l = [(si, gi, grp)
                               for si in range(8)
                               for gi, grp in enumerate(groups)]
                    inflight = []
                    for u in units_l:
                        ex = emit_front(*u)
                        if len(inflight) >= 3:
                            pu, pex = inflight.pop(0)
                            emit_pv(*pu, ex=pex)
                        inflight.append((u, ex))
                    for pu, pex in inflight:
                        emit_pv(*pu, ex=pex)
                    flush_pending()
                    state["prev"] = (ci, y_c,
                                     ppsb_pool.tile([P, 4, 1024], f16,
                                                    name="ppsb"))

                # ---- B1: the 512-wide chunks, st bufs=3 --------------------
                with (
                    tc.tile_pool(name="st1", bufs=3, space="PSUM") as st1,
                    tc.tile_pool(name="yp1", bufs=2, space="PSUM") as y_pool,
                ):
                    def acc1(name):
                        t = st1.tile([P, 2, 512], f32, name=name, tag="st")
                        return t[:, 0, :]

                    units1 = make_units(acc1)
                    v_tile1 = units1[0]
                    qk_unit1 = units1[1]
                    for tt in range(4):
                        v_tile1(tt)
                    state["vdone"] = 4
                    qk_fill = [(qk_unit1, (2, m, n))
                               for m in range(2) for n in range(NQ)]
                    qk_fill += [(qk_unit1, (3, m, n))
                                for m in range(2) for n in range(NQ)]
                    process_chunk(0, st1, 512, units1, qk_fill)
                    process_chunk(1, st1, 512, units1, [])
                    process_chunk(2, st1, 512, units1, [])

                # ---- B2: the 256-wide chunks, 1-bank st tiles, bufs=4 ------
                with (
                    tc.tile_pool(name="st2", bufs=6, space="PSUM") as st2,
                    tc.tile_pool(name="yp2", bufs=2, space="PSUM") as y_pool,
                ):
                    def acc2(name):
                        t = st2.tile([P, 2, 256], f32, name=name, tag="st")
                        return t.rearrange("p a b -> p (a b)")

                    units2 = make_units(acc2)
                    process_chunk(3, st2, 256, units2, [])
                    process_chunk(4, st2, 256, units2, [])

                    # final chunk's partial projection (tail — kept small)
                    pci, py_c, ppp_sb = state["prev"]
                    for tt in range(CHUNKS[pci][1] // P):
                        for hf in range(2):
                            units2[2](pci, py_c, ppp_sb, tt, hf)
                    emit_rs()
                    for ci, (cq0, cw) in enumerate(CHUNKS):
                        nc.sync.dma_start(out=out[cq0:cq0 + cw, :],
                                          in_=rs_out[ci][:])
    nc.compile()
    return nc


def _make_runner(nc):
    """Reusable 8-core SPMD runner (jit built once)."""
    import jax
    from jax.sharding import Mesh, PartitionSpec
    from jax.experimental.shard_map import shard_map
    from concourse import bass2jax
    import concourse.mybir as mybir

    bass2jax.install_neuronx_cc_hook()
    partition_name = (nc.partition_id_tensor.name
                      if nc.partition_id_tensor else None)
    in_names, out_names, out_avals, zero_outs = [], [], [], []
    for alloc in nc.m.functions[0].allocations:
        if not isinstance(alloc, mybir.MemoryLocationSet):
            continue
        name = alloc.memorylocations[0].name
        if alloc.kind == "ExternalInput":
            if name != partition_name:
                in_names.append(name)
        elif alloc.kind == "ExternalOutput":
            shape = tuple(alloc.tensor_shape)
            dtype = mybir.dt.np(alloc.dtype)
            out_names.append(name)
            out_avals.append(jax.core.ShapedArray(shape, dtype))
            zero_outs.append(np.zeros(shape, dtype))
    n_params = len(in_names)
    n_outs = len(out_avals)
    all_in = list(in_names) + list(out_names)
    if partition_name is not None:
        all_in.append(partition_name)

    def _body(*args):
        operands = list(args)
        if partition_name is not None:
            operands.append(bass2jax.partition_id_tensor())
        outs = bass2jax._bass_exec_p.bind(
            *operands,
            out_avals=tuple(out_avals),
            in_names=tuple(all_in),
            out_names=tuple(out_names),
            lowering_input_output_aliases=(),
            sim_require_finite=True,
            sim_require_nnan=True,
            nc=nc,
        )
        return tuple(outs)

    devices = jax.devices()[:N_CORES]
    mesh = Mesh(np.asarray(devices), ("core",))
    in_specs = (PartitionSpec("core"),) * (n_params + n_outs)
    out_specs = (PartitionSpec("core"),) * n_outs
    donate = tuple(range(n_params, n_params + n_outs))
    sharded = jax.jit(
        shard_map(_body, mesh=mesh, in_specs=in_specs, out_specs=out_specs,
                  check_rep=False),
        donate_argnums=donate, keep_unused=True)

    def run(in_maps):
        per_core = [[np.asarray(m[k]) for k in in_names] for m in in_maps]
        concat_in = [
            np.concatenate([per_core[c][i] for c in range(N_CORES)], axis=0)
            for i in range(n_params)]
        concat_zeros = [
            np.zeros((N_CORES * z.shape[0], *z.shape[1:]), z.dtype)
            for z in zero_outs]
        outs = sharded(*concat_in, *concat_zeros)
        jax.block_until_ready(outs)
        return [
            {name: np.asarray(outs[i]).reshape(N_CORES, *out_avals[i].shape)[c]
             for i, name in enumerate(out_names)}
            for c in range(N_CORES)]

    return run


def kernel(x, w_qkv, b_qkv, w_proj, b_proj):
    x = np.asarray(x, dtype=np.float32)
    w_qkv = np.asarray(w_qkv, dtype=np.float32)
    b_qkv = np.asarray(b_qkv, dtype=np.float32)
    w_proj = np.asarray(w_proj, dtype=np.float32)
    b_proj = np.asarray(b_proj, dtype=np.float32)

    w_q, w_k, w_v = w_qkv[0:1024], w_qkv[1024:2048], w_qkv[2048:3072]
    b_q, b_k, b_v = b_qkv[0:1024], b_qkv[1024:2048], b_qkv[2048:3072]
    has_qk_bias = bool(np.any(b_q) or np.any(b_k))

    key = ("runner", has_qk_bias)
    if key not in _RUNNER_CACHE:
        nc = _build(has_qk_bias)
        _RUNNER_CACHE[key] = _make_runner(nc)
    run = _RUNNER_CACHE[key]

    # causal mask lookup: tri[k, m] = 1.0 iff k <= m - 384
    kk = np.arange(P)[:, None]
    mm = np.arange(896)[None, :]
    tri = (kk <= mm - 384).astype(np.float16)

    in_maps = []
    for core in range(N_CORES):
        b, g = divmod(core, 2)
        xT_c = np.ascontiguousarray(x[b].T).astype(np.float16)  # [1024, 2048]
        if has_qk_bias:
            xT_c = np.concatenate(
                [xT_c, np.ones((1, T), np.float16)], axis=0)
        wqk_c = np.empty((D_MODEL + (1 if has_qk_bias else 0), 1024),
                         np.float16)
        for p in range(PAIRS):
            hA = 8 * g + 2 * p
            hB = hA + 1
            cols = p * 256
            wqk_c[:D_MODEL, cols + 0:cols + 64] = w_q[hA * 64:(hA + 1) * 64].T
            wqk_c[:D_MODEL, cols + 64:cols + 128] = w_q[hB * 64:(hB + 1) * 64].T
            wqk_c[:D_MODEL, cols + 128:cols + 192] = w_k[hA * 64:(hA + 1) * 64].T
            wqk_c[:D_MODEL, cols + 192:cols + 256] = w_k[hB * 64:(hB + 1) * 64].T
            if has_qk_bias:
                wqk_c[D_MODEL, cols + 0:cols + 64] = b_q[hA * 64:(hA + 1) * 64]
                wqk_c[D_MODEL, cols + 64:cols + 128] = b_q[hB * 64:(hB + 1) * 64]
                wqk_c[D_MODEL, cols + 128:cols + 192] = b_k[hA * 64:(hA + 1) * 64]
                wqk_c[D_MODEL, cols + 192:cols + 256] = b_k[hB * 64:(hB + 1) * 64]
        wv_c = np.ascontiguousarray(
            w_v[8 * g * 64:(8 * g + 8) * 64].T).astype(np.float16)
        # wp rows = this core's 512 y-columns (its 8 heads), all 1024 outputs
        wp_c = np.ascontiguousarray(
            w_proj.T[g * 512:(g + 1) * 512, :]).astype(np.float16)
        in_maps.append({
            "xT": xT_c, "wqk": wqk_c, "wv": wv_c, "wp": wp_c, "tri": tri,
        })

    results = run(in_maps)

    out = np.empty((B, T, D_MODEL), dtype=np.float32)
    for core in range(N_CORES):
        b, g = divmod(core, 2)
        out[b, :, g * 512:(g + 1) * 512] = results[core]["out"].astype(
            np.float32)

    # exact host-side bias folds (v-bias rides softmax row-sums == 1;
    # proj bias is additive)
    if np.any(b_v):
        out += (b_v @ w_proj.T)[None, None, :]
    if np.any(b_proj):
        out += b_proj[None, None, :]
    return out
